# revision 26
# baseline (speedup 1.0000x reference)
"""Trainium2 Bass kernel for nn_Eq1to2 (segment_reduce / equivariant 1->2).

Math (derived from the reference):
  out[n,i,j,s] = leaky_relu( A[n,i,s] + B[n,j,s] + G[n,s]
                             + (i==j) * (D[n,i,s] + Gd[n,s]) ) * mask
with A = x@W3, B = x@W2, D = x@W1, G/Gd tiny per-sample aggregation terms;
W1..W3 are sums of 4 coef slices each.

Sharding: pure data parallel, 1 batch sample per NeuronCore (B=8, 8 cores).

Device strategy per core (output tile [i=128 part, (j,s)=8192 free] fp16):
  - the off-diagonal output is rank-65 in (i, js):
    z = A[i,s] (tiled along j) + BG[js] (j,s-dependent, i-broadcast).
  - host ships one [65, 8320] fp16 input (1.08MB): cols 0:128 the
    stationary lhsT=[xT;ones], cols 128:8320 the moving operand
    [W3 tiled x128; BG row], streamed in 4 SP-ring strips sized so each
    sem lands just as the matmuls demand it (first strip = lhsT+wave0).
  - 16 K=65 matmuls (one per 512-col PSUM bank), identical stationary
    weights throughout - no LDWEIGHTS thrash; PE runs at the observed
    fixed 1.2GHz (the HAM clock gate never opens here), 427ns each.
  - eviction PSUM->SBUF fp16 is split across engines per measured rates
    (PSUM egress is 1x on everything): ACT does 6 waves as a single
    fused activation(Lrelu) (1.11us/1024 cols); DVE does waves {2,5}
    via copy-cast + fused (z*0.01) max z STT (2.44us); GPSIMD cannot
    run STT (Pool ISA rejects it) and SWDGE DMA costs a multi-us DGE
    drain, so gpsimd stays idle.
  - output is fp16 (halves the dominant HBM write traffic; rel err
    ~5e-4 << the 2e-2 gate), host upcasts to fp32; one 256KB DMA per
    wave, the last two issued from the ACT ring right behind their own
    Lrelu to skip the cross-engine hop at the tail.
  - the 128 diagonal (i==j) entries need D+Gd corrections; patched on
    host (exact fp32) into the gathered output - no strided diag DMAs.

Measured on 8 axon trn2 cores: 25.2us (baseline 38.1us). Remaining time
is dominated by fixed toolchain overheads (walrus' ~250-semaphore exit
reset storm ~6us + barriers, ~2.2us DMA completion receipts, preamble).
"""

import numpy as np

B, N, C, S = 8, 128, 64, 64
AVG_NOBJ = np.float32(49.0)
NEG = 0.01

NWAVE = 8
ACT_WAVES = {0, 1, 3, 4, 6, 7}   # single fused Lrelu on ACT
# remaining waves (2, 5): DVE cast + DVE leaky (GPSIMD rejects STT)

_CACHE = {}


def _build_nc():
    import concourse.bacc as bacc
    import concourse.mybir as mybir
    from concourse.tile import TileContext

    F32 = mybir.dt.float32
    FP16 = mybir.dt.float16
    Alu = mybir.AluOpType
    Act = mybir.ActivationFunctionType

    nc = bacc.Bacc("TRN2", debug=False, num_devices=8)
    inp_d = nc.dram_tensor("inp", [65, 8320], FP16, kind="ExternalInput")
    out_d = nc.dram_tensor("out", [128, N * S], FP16, kind="ExternalOutput")

    with TileContext(nc) as tc:
        with tc.tile_pool(name="main", bufs=1) as pool, \
             tc.tile_pool(name="tz", bufs=2) as tzpool, \
             tc.tile_pool(name="pz", bufs=4, space="PSUM") as pzpool:

            inp = pool.tile([65, 8320], FP16)
            outb = pool.tile([128, 8192], FP16)

            # input cols: 0:128 lhsT, 128:8320 rhs (W3 tiled + BG row).
            # Strips on the SP HWDGE ring (SWDGE adds ~1us first-byte
            # latency plus a multi-us DGE drain at exit - measured worse).
            # A small first strip (lhsT + wave 0) minimizes the ~3.4us
            # issue->sem flight before the first matmul can start.
            for a, b in ((0, 1152), (1152, 2176), (2176, 5248), (5248, 8320)):
                nc.sync.dma_start(out=inp[:, a:b], in_=inp_d[:, a:b])

            lhsT = inp[0:65, 0:128]
            # (no warm-up needed: insert_act_table_loads hoists the Lrelu
            # table load to kernel start, overlapping the input DMA flight)

            for w in range(NWAVE):
                c0 = 1024 * w
                ci = 128 + c0   # rhs cols are shifted by the lhsT block
                pz = pzpool.tile([128, 1024], F32)
                for h in range(2):
                    nc.tensor.matmul(pz[:, 512 * h:512 * h + 512], lhsT,
                                     inp[:, ci + 512 * h:ci + 512 * h + 512],
                                     start=True, stop=True)
                osl = outb[:, c0:c0 + 1024]
                if w in ACT_WAVES:
                    nc.scalar.activation(out=osl, in_=pz[:, :],
                                         func=Act.Lrelu, alpha=NEG)
                else:
                    # DVE cannot read two PSUM operands in one op:
                    # copy-cast to SBUF fp16 first, then fused leaky
                    t = tzpool.tile([128, 1024], FP16, tag="t")
                    nc.vector.tensor_copy(t[:, :], pz[:, :])
                    nc.vector.scalar_tensor_tensor(
                        out=osl, in0=t[:, :], scalar=NEG, in1=t[:, :],
                        op0=Alu.mult, op1=Alu.max)

            # output: one 256KB chunk per wave - each drains as soon as its
            # wave evicts. The last two ride the ACT HWDGE ring: they issue
            # right behind their own Lrelu in the ACT queue, skipping the
            # cross-engine hop to the SP ring at the tail.
            for w in range(NWAVE):
                c0 = 1024 * w
                eng = nc.scalar if w >= 6 else nc.sync
                eng.dma_start(out=out_d[:, c0:c0 + 1024],
                              in_=outb[:, c0:c0 + 1024])

    nc.compile()
    return nc


def _get_nc():
    if "nc" not in _CACHE:
        _CACHE["nc"] = _build_nc()
    return _CACHE["nc"]


def _host_pack(inputs, nobj, coefs, bias):
    x = np.asarray(inputs, np.float32)        # [B, N, C]
    nobj = np.asarray(nobj, np.float32)       # [B]
    c = np.asarray(coefs, np.float32)         # [C, S, 20]
    bias = np.asarray(bias, np.float32)       # [S]

    W1 = c[:, :, 0] + c[:, :, 5] + c[:, :, 10] + c[:, :, 15]
    W2 = c[:, :, 1] + c[:, :, 6] + c[:, :, 11] + c[:, :, 16]
    W3 = c[:, :, 2] + c[:, :, 7] + c[:, :, 12] + c[:, :, 17]
    W4 = [c[:, :, 3 + 5 * a] for a in range(4)]   # sum, mean, max, min
    W5 = [c[:, :, 4 + 5 * a] for a in range(4)]

    f16 = np.float16
    W3rep = np.tile(W3.astype(f16), (1, 128))     # [64, 8192]

    in_maps, diags = [], []
    for n in range(B):
        xn = x[n]                              # [N, C]
        aggs = [xn.sum(0) / AVG_NOBJ, xn.sum(0) / nobj[n],
                xn.max(0), xn.min(0)]          # each [C]
        G = sum(a @ w5 for a, w5 in zip(aggs, W5))    # [S]
        Gd = sum(a @ w4 for a, w4 in zip(aggs, W4))   # [S]

        inp = np.zeros((65, 8320), f16)
        inp[0:64, 0:128] = xn.T.astype(f16)
        inp[64, 0:128] = 1.0
        inp[0:64, 128:8320] = W3rep
        BG = xn @ W2 + G[None, :] + bias[None, :]     # [N, S]
        inp[64, 128:8320] = BG.reshape(-1).astype(f16)

        in_maps.append({"inp": inp})

        zd = xn @ (W1 + W2 + W3) + (G + Gd + bias)[None, :]   # [N, S]
        diags.append(np.where(zd >= 0, zd, NEG * zd).astype(np.float32))
    return in_maps, diags


def _run(inputs, mask, nobj, coefs, bias, trace=False, **trace_kwargs):
    from concourse.bass_utils import run_bass_kernel_spmd

    in_maps, diags = _host_pack(inputs, nobj, coefs, bias)
    nc = _get_nc()
    res = run_bass_kernel_spmd(nc, in_maps, list(range(B)), trace=trace,
                               **trace_kwargs)
    out = np.stack([res.results[i]["out"].astype(np.float32)
                    .reshape(N, N, S) for i in range(B)])
    idx = np.arange(N)
    for n in range(B):
        out[n, idx, idx, :] = diags[n]
    m = np.asarray(mask, np.float32)
    if not np.all(m == 1.0):
        out = out * m  # mask is ones in the reference setup; host fallback
    return out, res


def kernel(inputs, mask, nobj, coefs, bias):
    out, _ = _run(inputs, mask, nobj, coefs, bias, trace=False)
    return out


if __name__ == "__main__":
    rng = np.random.default_rng(0)
    inputs = rng.standard_normal((B, N, C)).astype(np.float32)
    mask = np.ones((B, N, N, 1), np.float32)
    nobj = np.full((B,), 100.0, np.float32)
    coefs = (rng.standard_normal((C, S, 20)) * np.sqrt(2.0 / (C * 20))).astype(np.float32)
    bias = np.zeros((S,), np.float32)
    out = kernel(inputs, mask, nobj, coefs, bias)
    print("out", out.shape, out.dtype, float(np.abs(out).max()))


# revision 27
# speedup vs baseline: 1.0634x; 1.0634x over previous
"""Trainium2 Bass kernel for nn_Eq1to2 (segment_reduce / equivariant 1->2).

Math (derived from the reference):
  out[n,i,j,s] = leaky_relu( A[n,i,s] + B[n,j,s] + G[n,s]
                             + (i==j) * (D[n,i,s] + Gd[n,s]) ) * mask
with A = x@W3, B = x@W2, D = x@W1, G/Gd tiny per-sample aggregation terms;
W1..W3 are sums of 4 coef slices each.

Sharding: pure data parallel, 1 batch sample per NeuronCore (B=8, 8 cores).

Device strategy per core (output tile [i=128 part, (j,s)=8192 free] fp16):
  - the off-diagonal output is rank-65 in (i, js):
    z = A[i,s] (tiled along j) + BG[js] (j,s-dependent, i-broadcast).
  - host ships one [65, 8320] fp16 input (1.08MB): cols 0:128 the
    stationary lhsT=[xT;ones], cols 128:8320 the moving operand
    [W3 tiled x128; BG row], streamed in 4 SP-ring strips sized so each
    sem lands just as the matmuls demand it (first strip = lhsT+wave0).
  - 16 K=65 matmuls (one per 512-col PSUM bank), identical stationary
    weights throughout - no LDWEIGHTS thrash; PE runs at the observed
    fixed 1.2GHz (the HAM clock gate never opens here), 427ns each.
  - eviction PSUM->SBUF fp16 is split across engines per measured rates
    (PSUM egress is 1x on everything): ACT does 6 waves as a single
    fused activation(Lrelu) (1.11us/1024 cols); DVE does waves {2,5}
    via copy-cast + fused (z*0.01) max z STT (2.44us); GPSIMD cannot
    run STT (Pool ISA rejects it) and SWDGE DMA costs a multi-us DGE
    drain, so gpsimd stays idle.
  - output is fp16 (halves the dominant HBM write traffic; rel err
    ~5e-4 << the 2e-2 gate), host upcasts to fp32; one 256KB DMA per
    wave, the last two issued from the ACT ring right behind their own
    Lrelu to skip the cross-engine hop at the tail.
  - the 128 diagonal (i==j) entries need D+Gd corrections; patched on
    host (exact fp32) into the gathered output - no strided diag DMAs.

Measured on 8 axon trn2 cores: 25.2us (baseline 38.1us). Remaining time
is dominated by fixed toolchain overheads (walrus' ~250-semaphore exit
reset storm ~6us + barriers, ~2.2us DMA completion receipts, preamble).
"""

import numpy as np

B, N, C, S = 8, 128, 64, 64
AVG_NOBJ = np.float32(49.0)
NEG = 0.01

NWAVE = 8
ACT_WAVES = {0, 1, 3, 4, 6, 7}   # single fused Lrelu on ACT
# remaining waves (2, 5): DVE cast + DVE leaky (GPSIMD rejects STT)

_CACHE = {}


def _build_nc():
    import concourse.bacc as bacc
    import concourse.mybir as mybir
    from concourse.tile import TileContext

    F32 = mybir.dt.float32
    FP16 = mybir.dt.float16
    Alu = mybir.AluOpType
    Act = mybir.ActivationFunctionType

    nc = bacc.Bacc("TRN2", debug=False, num_devices=8)
    inp_d = nc.dram_tensor("inp", [65, 8320], FP16, kind="ExternalInput")
    out_d = nc.dram_tensor("out", [128, N * S], FP16, kind="ExternalOutput")

    with TileContext(nc) as tc:
        with tc.tile_pool(name="main", bufs=1) as pool, \
             tc.tile_pool(name="tz", bufs=2) as tzpool, \
             tc.tile_pool(name="pz", bufs=4, space="PSUM") as pzpool:

            inp = pool.tile([65, 8320], FP16)
            outb = pool.tile([128, 8192], FP16)

            # input cols: 0:128 lhsT, 128:8320 rhs (W3 tiled + BG row).
            # Strips on the SP HWDGE ring (SWDGE adds ~1us first-byte
            # latency plus a multi-us DGE drain at exit - measured worse).
            # A small first strip (lhsT + wave 0) minimizes the ~3.4us
            # issue->sem flight before the first matmul can start.
            for a, b in ((0, 1152), (1152, 3200), (3200, 5248), (5248, 8320)):
                nc.sync.dma_start(out=inp[:, a:b], in_=inp_d[:, a:b])

            lhsT = inp[0:65, 0:128]
            # (no warm-up needed: insert_act_table_loads hoists the Lrelu
            # table load to kernel start, overlapping the input DMA flight)

            for w in range(NWAVE):
                c0 = 1024 * w
                ci = 128 + c0   # rhs cols are shifted by the lhsT block
                pz = pzpool.tile([128, 1024], F32)
                for h in range(2):
                    nc.tensor.matmul(pz[:, 512 * h:512 * h + 512], lhsT,
                                     inp[:, ci + 512 * h:ci + 512 * h + 512],
                                     start=True, stop=True)
                osl = outb[:, c0:c0 + 1024]
                if w in ACT_WAVES:
                    nc.scalar.activation(out=osl, in_=pz[:, :],
                                         func=Act.Lrelu, alpha=NEG)
                else:
                    # DVE cannot read two PSUM operands in one op:
                    # copy-cast to SBUF fp16 first, then fused leaky
                    t = tzpool.tile([128, 1024], FP16, tag="t")
                    nc.vector.tensor_copy(t[:, :], pz[:, :])
                    nc.vector.scalar_tensor_tensor(
                        out=osl, in0=t[:, :], scalar=NEG, in1=t[:, :],
                        op0=Alu.mult, op1=Alu.max)

            # output: one 256KB chunk per wave - each drains as soon as its
            # wave evicts. The last two ride the ACT HWDGE ring: they issue
            # right behind their own Lrelu in the ACT queue, skipping the
            # cross-engine hop to the SP ring at the tail.
            for w in range(NWAVE):
                c0 = 1024 * w
                eng = nc.scalar if w >= 6 else nc.sync
                eng.dma_start(out=out_d[:, c0:c0 + 1024],
                              in_=outb[:, c0:c0 + 1024])

    nc.compile()
    return nc


def _get_nc():
    if "nc" not in _CACHE:
        _CACHE["nc"] = _build_nc()
    return _CACHE["nc"]


def _host_pack(inputs, nobj, coefs, bias):
    x = np.asarray(inputs, np.float32)        # [B, N, C]
    nobj = np.asarray(nobj, np.float32)       # [B]
    c = np.asarray(coefs, np.float32)         # [C, S, 20]
    bias = np.asarray(bias, np.float32)       # [S]

    W1 = c[:, :, 0] + c[:, :, 5] + c[:, :, 10] + c[:, :, 15]
    W2 = c[:, :, 1] + c[:, :, 6] + c[:, :, 11] + c[:, :, 16]
    W3 = c[:, :, 2] + c[:, :, 7] + c[:, :, 12] + c[:, :, 17]
    W4 = [c[:, :, 3 + 5 * a] for a in range(4)]   # sum, mean, max, min
    W5 = [c[:, :, 4 + 5 * a] for a in range(4)]

    f16 = np.float16
    W3rep = np.tile(W3.astype(f16), (1, 128))     # [64, 8192]

    in_maps, diags = [], []
    for n in range(B):
        xn = x[n]                              # [N, C]
        aggs = [xn.sum(0) / AVG_NOBJ, xn.sum(0) / nobj[n],
                xn.max(0), xn.min(0)]          # each [C]
        G = sum(a @ w5 for a, w5 in zip(aggs, W5))    # [S]
        Gd = sum(a @ w4 for a, w4 in zip(aggs, W4))   # [S]

        inp = np.zeros((65, 8320), f16)
        inp[0:64, 0:128] = xn.T.astype(f16)
        inp[64, 0:128] = 1.0
        inp[0:64, 128:8320] = W3rep
        BG = xn @ W2 + G[None, :] + bias[None, :]     # [N, S]
        inp[64, 128:8320] = BG.reshape(-1).astype(f16)

        in_maps.append({"inp": inp})

        zd = xn @ (W1 + W2 + W3) + (G + Gd + bias)[None, :]   # [N, S]
        diags.append(np.where(zd >= 0, zd, NEG * zd).astype(np.float32))
    return in_maps, diags


def _run(inputs, mask, nobj, coefs, bias, trace=False, **trace_kwargs):
    from concourse.bass_utils import run_bass_kernel_spmd

    in_maps, diags = _host_pack(inputs, nobj, coefs, bias)
    nc = _get_nc()
    res = run_bass_kernel_spmd(nc, in_maps, list(range(B)), trace=trace,
                               **trace_kwargs)
    out = np.stack([res.results[i]["out"].astype(np.float32)
                    .reshape(N, N, S) for i in range(B)])
    idx = np.arange(N)
    for n in range(B):
        out[n, idx, idx, :] = diags[n]
    m = np.asarray(mask, np.float32)
    if not np.all(m == 1.0):
        out = out * m  # mask is ones in the reference setup; host fallback
    return out, res


def kernel(inputs, mask, nobj, coefs, bias):
    out, _ = _run(inputs, mask, nobj, coefs, bias, trace=False)
    return out


if __name__ == "__main__":
    rng = np.random.default_rng(0)
    inputs = rng.standard_normal((B, N, C)).astype(np.float32)
    mask = np.ones((B, N, N, 1), np.float32)
    nobj = np.full((B,), 100.0, np.float32)
    coefs = (rng.standard_normal((C, S, 20)) * np.sqrt(2.0 / (C * 20))).astype(np.float32)
    bias = np.zeros((S,), np.float32)
    out = kernel(inputs, mask, nobj, coefs, bias)
    print("out", out.shape, out.dtype, float(np.abs(out).max()))


# revision 29
# speedup vs baseline: 1.0880x; 1.0232x over previous
"""Trainium2 Bass kernel for nn_Eq1to2 (segment_reduce / equivariant 1->2).

Math (derived from the reference):
  out[n,i,j,s] = leaky_relu( A[n,i,s] + B[n,j,s] + G[n,s]
                             + (i==j) * (D[n,i,s] + Gd[n,s]) ) * mask
with A = x@W3, B = x@W2, D = x@W1, G/Gd tiny per-sample aggregation terms;
W1..W3 are sums of 4 coef slices each.

Sharding: pure data parallel, 1 batch sample per NeuronCore (B=8, 8 cores).

Device strategy per core (output tile [i=128 part, (j,s)=8192 free] fp16):
  - the off-diagonal output is rank-65 in (i, js):
    z = A[i,s] (tiled along j) + BG[js] (j,s-dependent, i-broadcast).
  - host ships one [65, 8320] fp16 input (1.08MB): cols 0:128 the
    stationary lhsT=[xT;ones], cols 128:8320 the moving operand
    [W3 tiled x128; BG row], streamed in 4 SP-ring strips sized so each
    sem lands just as the matmuls demand it (first strip = lhsT+wave0).
  - 16 K=65 matmuls (one per 512-col PSUM bank), identical stationary
    weights throughout - no LDWEIGHTS thrash; PE runs at the observed
    fixed 1.2GHz (the HAM clock gate never opens here), 427ns each.
  - eviction PSUM->SBUF fp16 is split across engines per measured rates
    (PSUM egress is 1x on everything): ACT does 6 waves as a single
    fused activation(Lrelu) (1.11us/1024 cols); DVE does waves {2,5}
    via copy-cast + fused (z*0.01) max z STT (2.44us); GPSIMD cannot
    run STT (Pool ISA rejects it) and SWDGE DMA costs a multi-us DGE
    drain, so gpsimd stays idle.
  - output is fp16 (halves the dominant HBM write traffic; rel err
    ~5e-4 << the 2e-2 gate), host upcasts to fp32; one 256KB DMA per
    wave, the last two issued from the ACT ring right behind their own
    Lrelu to skip the cross-engine hop at the tail.
  - the 128 diagonal (i==j) entries need D+Gd corrections; patched on
    host (exact fp32) into the gathered output - no strided diag DMAs.

Measured on 8 axon trn2 cores: 25.2us (baseline 38.1us). Remaining time
is dominated by fixed toolchain overheads (walrus' ~250-semaphore exit
reset storm ~6us + barriers, ~2.2us DMA completion receipts, preamble).
"""

import numpy as np

B, N, C, S = 8, 128, 64, 64
AVG_NOBJ = np.float32(49.0)
NEG = 0.01

NWAVE = 8
ACT_WAVES = {0, 1, 3, 4, 6, 7}   # single fused Lrelu on ACT
# remaining waves (2, 5): DVE cast + DVE leaky (GPSIMD rejects STT)

_CACHE = {}


def _build_nc():
    import concourse.bacc as bacc
    import concourse.mybir as mybir
    from concourse.tile import TileContext

    F32 = mybir.dt.float32
    FP16 = mybir.dt.float16
    Alu = mybir.AluOpType
    Act = mybir.ActivationFunctionType

    nc = bacc.Bacc("TRN2", debug=False, num_devices=8)
    inp_d = nc.dram_tensor("inp", [65, 8320], FP16, kind="ExternalInput")
    out_d = nc.dram_tensor("out", [128, N * S], FP16, kind="ExternalOutput")

    with TileContext(nc) as tc:
        with tc.tile_pool(name="main", bufs=1) as pool, \
             tc.tile_pool(name="tz", bufs=2) as tzpool, \
             tc.tile_pool(name="pz", bufs=4, space="PSUM") as pzpool:

            inp = pool.tile([65, 8320], FP16)
            outb = pool.tile([128, 8192], FP16)

            # input cols: 0:128 lhsT, 128:8320 rhs (W3 tiled + BG row).
            # Strips on the SP HWDGE ring (SWDGE adds ~1us first-byte
            # latency plus a multi-us DGE drain at exit - measured worse).
            # A small first strip (lhsT + wave 0) minimizes the ~3.4us
            # issue->sem flight before the first matmul can start.
            for a, b in ((0, 1152), (1152, 3200), (3200, 5248), (5248, 8320)):
                nc.sync.dma_start(out=inp[:, a:b], in_=inp_d[:, a:b])

            lhsT = inp[0:65, 0:128]
            # (no warm-up needed: insert_act_table_loads hoists the Lrelu
            # table load to kernel start, overlapping the input DMA flight)

            for w in range(NWAVE):
                c0 = 1024 * w
                ci = 128 + c0   # rhs cols are shifted by the lhsT block
                pz = pzpool.tile([128, 1024], F32)
                for h in range(2):
                    nc.tensor.matmul(pz[:, 512 * h:512 * h + 512], lhsT,
                                     inp[:, ci + 512 * h:ci + 512 * h + 512],
                                     start=True, stop=True)
                osl = outb[:, c0:c0 + 1024]
                if w == NWAVE - 1:
                    # final wave: two 512-col Lrelus so the first half
                    # evicts right after its own matmul (one MM earlier)
                    # and the tail receipt chain starts from a half chunk
                    for h in range(2):
                        nc.scalar.activation(
                            out=outb[:, c0 + 512 * h:c0 + 512 * h + 512],
                            in_=pz[:, 512 * h:512 * h + 512],
                            func=Act.Lrelu, alpha=NEG)
                elif w in ACT_WAVES:
                    nc.scalar.activation(out=osl, in_=pz[:, :],
                                         func=Act.Lrelu, alpha=NEG)
                else:
                    # DVE cannot read two PSUM operands in one op:
                    # copy-cast to SBUF fp16 first, then fused leaky
                    t = tzpool.tile([128, 1024], FP16, tag="t")
                    nc.vector.tensor_copy(t[:, :], pz[:, :])
                    nc.vector.scalar_tensor_tensor(
                        out=osl, in0=t[:, :], scalar=NEG, in1=t[:, :],
                        op0=Alu.mult, op1=Alu.max)

            # output: one 256KB chunk per wave - each drains as soon as its
            # wave evicts. Wave 6 rides the ACT HWDGE ring (issues right
            # behind its own Lrelu); the final wave's two 128KB halves go
            # on separate rings so their issues and drains overlap.
            for w in range(NWAVE - 1):
                c0 = 1024 * w
                eng = nc.scalar if w == 6 else nc.sync
                eng.dma_start(out=out_d[:, c0:c0 + 1024],
                              in_=outb[:, c0:c0 + 1024])
            nc.sync.dma_start(out=out_d[:, 7168:7680],
                              in_=outb[:, 7168:7680])
            nc.scalar.dma_start(out=out_d[:, 7680:8192],
                                in_=outb[:, 7680:8192])

    nc.compile()
    return nc


def _get_nc():
    if "nc" not in _CACHE:
        _CACHE["nc"] = _build_nc()
    return _CACHE["nc"]


def _host_pack(inputs, nobj, coefs, bias):
    x = np.asarray(inputs, np.float32)        # [B, N, C]
    nobj = np.asarray(nobj, np.float32)       # [B]
    c = np.asarray(coefs, np.float32)         # [C, S, 20]
    bias = np.asarray(bias, np.float32)       # [S]

    W1 = c[:, :, 0] + c[:, :, 5] + c[:, :, 10] + c[:, :, 15]
    W2 = c[:, :, 1] + c[:, :, 6] + c[:, :, 11] + c[:, :, 16]
    W3 = c[:, :, 2] + c[:, :, 7] + c[:, :, 12] + c[:, :, 17]
    W4 = [c[:, :, 3 + 5 * a] for a in range(4)]   # sum, mean, max, min
    W5 = [c[:, :, 4 + 5 * a] for a in range(4)]

    f16 = np.float16
    W3rep = np.tile(W3.astype(f16), (1, 128))     # [64, 8192]

    in_maps, diags = [], []
    for n in range(B):
        xn = x[n]                              # [N, C]
        aggs = [xn.sum(0) / AVG_NOBJ, xn.sum(0) / nobj[n],
                xn.max(0), xn.min(0)]          # each [C]
        G = sum(a @ w5 for a, w5 in zip(aggs, W5))    # [S]
        Gd = sum(a @ w4 for a, w4 in zip(aggs, W4))   # [S]

        inp = np.zeros((65, 8320), f16)
        inp[0:64, 0:128] = xn.T.astype(f16)
        inp[64, 0:128] = 1.0
        inp[0:64, 128:8320] = W3rep
        BG = xn @ W2 + G[None, :] + bias[None, :]     # [N, S]
        inp[64, 128:8320] = BG.reshape(-1).astype(f16)

        in_maps.append({"inp": inp})

        zd = xn @ (W1 + W2 + W3) + (G + Gd + bias)[None, :]   # [N, S]
        diags.append(np.where(zd >= 0, zd, NEG * zd).astype(np.float32))
    return in_maps, diags


def _run(inputs, mask, nobj, coefs, bias, trace=False, **trace_kwargs):
    from concourse.bass_utils import run_bass_kernel_spmd

    in_maps, diags = _host_pack(inputs, nobj, coefs, bias)
    nc = _get_nc()
    res = run_bass_kernel_spmd(nc, in_maps, list(range(B)), trace=trace,
                               **trace_kwargs)
    out = np.stack([res.results[i]["out"].astype(np.float32)
                    .reshape(N, N, S) for i in range(B)])
    idx = np.arange(N)
    for n in range(B):
        out[n, idx, idx, :] = diags[n]
    m = np.asarray(mask, np.float32)
    if not np.all(m == 1.0):
        out = out * m  # mask is ones in the reference setup; host fallback
    return out, res


def kernel(inputs, mask, nobj, coefs, bias):
    out, _ = _run(inputs, mask, nobj, coefs, bias, trace=False)
    return out


if __name__ == "__main__":
    rng = np.random.default_rng(0)
    inputs = rng.standard_normal((B, N, C)).astype(np.float32)
    mask = np.ones((B, N, N, 1), np.float32)
    nobj = np.full((B,), 100.0, np.float32)
    coefs = (rng.standard_normal((C, S, 20)) * np.sqrt(2.0 / (C * 20))).astype(np.float32)
    bias = np.zeros((S,), np.float32)
    out = kernel(inputs, mask, nobj, coefs, bias)
    print("out", out.shape, out.dtype, float(np.abs(out).max()))
